# revision 34
# baseline (speedup 1.0000x reference)
"""Trainium2 Bass kernel for nn_BICEPNeuralLayer.

Math: the reference module (Euler-Maruyama SDE scan -> Conv1d over time ->
time-mean -> linear projection) is LINEAR in the noise tensor, so the whole
pipeline collapses algebraically:

  paths[t] = c_b * sum_s retain^(t-s) eps_s          (c_b = feedback_b*sqrt(dt))
  mean_t(conv(paths)) folds to per-timestep weights on eps:
     out[b] = (c_b/NS) * (Tsum @ A[b] - T0 @ L[b] - T2 @ F[b]) + bias
  A[b,i] = sum_s gA[s] noise[b,s,i],   gA[s] = (1-retain^(NS-s))/(1-retain)
  L[b,i] = sum_s retain^(NS-1-s) noise[b,s,i]
  F[b,i] = noise[b,0,i]
  Tsum = out_w @ (W0+W1+W2), T0 = out_w @ W0  (Wk = conv_w[:,:,k])
  bias  = out_w @ conv_b + out_b

The F term carries ~1e-6 of the output variance (Var A ~ 8e5, Var L ~ 1e2,
Var F ~ 1) and is dropped: ~1.1e-3 relative error against a 2e-2 gate, for
0.5 MB less HBM stream and 8 fewer matmuls per core. The bias vector rides
for free inside mcat16: chunk 7's rows 104-127 are feature padding
(P=1000 -> 1024), so row 127 of mcat16[:,7,:] holds the bias and the host
injects noise values that make the matching V row come out as the scale
constant - the chunk-7 A-matmul then adds the bias with no extra
instruction or transfer.

Scale unification: the A path carries x2^12 (folded into the c broadcast)
and the L path lands at the same 2^12 (= SV*S8/2^4... SV=16, S8=256), so
ps_out and ps_lf are directly addable: the whole epilogue is ONE DVE
tensor_add into fp16, and the host divides by 4096 during the upcast.

Device work per core (pure data parallel over batch, 32 samples/core):
  The noise shard is pre-transposed on the host to chunk-major layout
  [q][s][b][i] so every DMA descriptor is an 8 KB sequential DRAM run, and
  rides eight 1 MB transfers on the sync HWDGE queue (slicing transfers
  finer exposes a per-transfer completion-receipt stall of the SDMA
  engines - measured 16-130 GB/s on 256 KB b-sliced quarters vs 420 GB/s
  on 1 MB chunks). All weights (one [128,256] fp16 consts block + the two
  mcat halves each) stream concurrently on the scalar HWDGE queue, so
  their receipts hide under the noise stream and every weight is resident
  by ~14 us.

  HAM discipline: PE_HAM clock-gates the PE to 1.2 GHz after any ~3.4 us
  activity window containing idle. The PE is kept continuously busy with
  zero-operand filler matmuls: a pre-stream burst covering the first
  chunk's DMA window, and a few per chunk (the PE at full clock is ~1.8x
  faster than the noise stream).

  Stage-1 writes alternate full PSUM banks per half-chunk: the DVE V-build
  reads bank k while the PE writes bank k+1 (sharing one bank serialized
  the engines and cost ~0.5 us per half, measured).

  per chunk q (software-pipelined: s1(q,h0), s2(q-1), s1(q,h1)):
    stage 1: 16 matmuls per half, lhsT=noise[q][:,b,:] (fp16, FWL)
             rhs=g2[128,2] -> psum[i, (b,{A,L})]   (~33 ns/matmul)
    V build: DVE reorder (b,v)->(v,b) fused with the per-sample feedback
             scale (A column pre-scaled x2^12), per half
    ACT:     fp8 copy (x16) of the L columns, per half
    stage 2: A-term fp16 matmul into ps_out, L-term fp8xfp8 into ps_lf

  The output is stored fp16 at x4096 (host upcasts and descales).
"""

import sys

if "/opt/trn_rl_repo" not in sys.path:
    sys.path.insert(0, "/opt/trn_rl_repo")

from contextlib import ExitStack

import numpy as np

import concourse.bass as bass
import concourse.tile as tile
from concourse import mybir
from concourse.bass_utils import run_bass_kernel_spmd

B, IN, OUT, P, NS = 256, 1024, 512, 1000, 128
NCORES = 8
BSH = B // NCORES  # 32 samples per core
NQ = 8             # feature chunks of 128 (P padded 1000 -> 1024)
PPAD = NQ * 128
NPRE = 12          # pre-stream HAM warmup fillers (dependency-free)

F32 = mybir.dt.float32
F16 = mybir.dt.float16
F8 = mybir.dt.float8e4
F16_NP = mybir.dt.np(F16)
F8_NP = mybir.dt.np(F8)
S8 = 256.0         # fp8 T0 slice scale
SV = 16.0          # fp8 V(L) scale
SA = S8 * SV       # = 4096: A-path scale, folded into the c broadcast

_CACHE = {}

LAST_RUN = None  # BassKernelResults of the most recent execution (for test.py)


def _split_sync_waits(nc: bass.Bass, max_waits: int = 1) -> int:
    """Walrus in this container accepts at most one sync-wait command per
    instruction. Tile emits instructions (notably the epilogue Drain and any
    op depending on two DMA queues) with several waits. Split the surplus
    onto single-wait NoOps inserted just before, on the same engine, which
    is semantically identical for sem-ge waits."""
    nid = 0
    for fn in nc.m.functions:
        for bb in fn.blocks:
            insts = list(bb.instructions)
            out, changed = [], False
            for inst in insts:
                si = inst.sync_info
                if si is not None and si.on_wait and len(si.on_wait) > max_waits:
                    waits = list(si.on_wait)
                    extra, keep = waits[:-max_waits], waits[-max_waits:]
                    for w in extra:
                        nid += 1
                        out.append(
                            mybir.InstNoOp(
                                name=f"waitsplit-{nid}",
                                sync_info=mybir.SyncInfo(on_wait=[w], on_update=[]),
                                bass_nofuse=True,
                                engine=inst.engine,
                            )
                        )
                    inst.sync_info = mybir.SyncInfo(
                        on_wait=keep, on_update=list(si.on_update)
                    )
                    changed = True
                out.append(inst)
            if changed:
                bb.instructions = out
    return nid


def _build_program() -> bass.Bass:
    if "nc" in _CACHE:
        return _CACHE["nc"]

    nc = bass.Bass()

    noise_d = nc.dram_tensor("noise_sh", [NQ, NS, BSH, 128], F16,
                             kind="ExternalInput")
    # cols 0:2 g2 (gA, gL), 2:66 c broadcast in (v,b) layout, rest pad
    cblk_d = nc.dram_tensor("cblk", [128, 256], F16, kind="ExternalInput")
    mcat16_d = nc.dram_tensor("mcat16", [128, NQ, OUT], F16, kind="ExternalInput")
    mcat8_d = nc.dram_tensor("mcat8", [128, NQ, OUT], F8, kind="ExternalInput")
    out_d = nc.dram_tensor("out", [BSH, OUT], F16, kind="ExternalOutput")

    with ExitStack() as ctx:
        tc = ctx.enter_context(tile.TileContext(nc))
        consts = ctx.enter_context(tc.tile_pool(name="consts", bufs=1))
        npool = ctx.enter_context(tc.tile_pool(name="noise", bufs=NQ))
        vpool = ctx.enter_context(tc.tile_pool(name="v", bufs=1))
        ps1 = ctx.enter_context(tc.tile_pool(name="ps1", bufs=4, space="PSUM"))
        ps2 = ctx.enter_context(tc.tile_pool(name="ps2", bufs=1, space="PSUM"))
        wps = ctx.enter_context(tc.tile_pool(name="wps", bufs=1, space="PSUM"))

        # ---- tiles ----
        cblk_sb = consts.tile([128, 256], F16, tag="cblk")
        mcat16_sb = consts.tile([128, NQ, OUT], F16, tag="mcat16")
        mcat8_sb = consts.tile([128, NQ, OUT], F8, tag="mcat8")
        noise_t = [npool.tile([NS, BSH, 128], F16, name=f"noise{q}", tag="noise")
                   for q in range(NQ)]
        g2_sb = cblk_sb[:, 0:2]
        cbc_sb = cblk_sb[:, 2:66].rearrange("p (v b) -> p v b", v=2)

        # ---- DMA issues first. Noise rides the sync queue as eight
        # uniform 1 MB transfers (uniform chunk cadence, no weight
        # bubbles); the weights stream concurrently on the scalar HWDGE
        # queue - slower, but the 2-deep stage-2 pipeline needs each mcat
        # piece only well after it lands.
        nc.scalar.dma_start(out=cblk_sb[:], in_=cblk_d[:])
        for q in range(NQ):
            nc.sync.dma_start(out=noise_t[q][:], in_=noise_d[q])
        nc.scalar.dma_start(out=mcat16_sb[:, 0:4, :], in_=mcat16_d[:][:, 0:4, :])
        nc.scalar.dma_start(out=mcat8_sb[:, 0:4, :], in_=mcat8_d[:][:, 0:4, :])
        nc.scalar.dma_start(out=mcat16_sb[:, 4:8, :], in_=mcat16_d[:][:, 4:8, :])
        nc.scalar.dma_start(out=mcat8_sb[:, 4:8, :], in_=mcat8_d[:][:, 4:8, :])

        # ---- HAM warmup/filler scaffolding: zero fp8 operands, scratch
        # psum bank. The pre-burst is dependency-free (the scheduler runs
        # it first); per-chunk fillers take a fake RAW dep on v8_t[q] so
        # the priority-heap scheduler slots them exactly into the PE's
        # per-chunk idle window (the PE outruns the noise stream ~1.5x,
        # and any idle re-throttles the PE clock to 1.2 GHz, doubling the
        # N=512 stage-2 matmuls and the runtime's epilogue barrier loop).
        warm_sb = consts.tile([128, 512], F16, tag="warm")
        nc.vector.memset(warm_sb[:], 0.0)
        warm_ps = wps.tile([128, 512], F32, tag="warmps")

        def filler(dep=None):
            lhsT = dep if dep is not None else warm_sb[:, 0:128]
            nc.tensor.matmul(warm_ps[0 : lhsT.free_size(), :], lhsT=lhsT,
                             rhs=warm_sb[:], start=True, stop=True)

        for _ in range(NPRE):
            filler()

        ps_out = ps2.tile([BSH, OUT], F32, tag="ps2")
        v_t = [vpool.tile([128, BSH], F16, name=f"v{q}", tag=f"v{q}")
               for q in range(NQ)]
        v8_t = [vpool.tile([128, BSH], F8, name=f"v8_{q}", tag=f"v8_{q}")
                for q in range(NQ)]
        HW_ = BSH // 2
        # one full PSUM bank per half-chunk (writing the bank the DVE is
        # still reading serializes PE vs DVE)
        pt_t = [ps1.tile([128, 512], F32, name=f"ps1_{qh}", tag="ps1")
                for qh in range(2 * NQ)]

        def stage1_half(q, h):
            pt = pt_t[2 * q + h]
            for j, b in enumerate(range(h * HW_, (h + 1) * HW_)):
                nc.tensor.matmul(
                    pt[:, j * 2 : j * 2 + 2],
                    lhsT=noise_t[q][:, b, :],
                    rhs=g2_sb,
                    start=True,
                    stop=True,
                )
            # psum -> V: two DVE ops straight off the psum, folding the
            # per-sample feedback scale c_b in. A path fp16 (x2^12 via the
            # c broadcast), L path fp8 (x16 via its own c columns).
            bs = slice(h * HW_, (h + 1) * HW_)
            src = pt[:, 0 : 2 * HW_].rearrange("p (b v) -> p v b", v=2)
            nc.vector.tensor_mul(v_t[q][:, bs], src[:, 0, :], cbc_sb[:, 0, bs])
            nc.vector.tensor_mul(v8_t[q][:, bs], src[:, 1, :], cbc_sb[:, 1, bs])

        def stage2(q):
            # both terms land at x2^12, so the L matmuls accumulate
            # straight into ps_out (fp8 and fp16 matmuls share the f32
            # accumulation); A first - it only needs the first DVE op
            nc.tensor.matmul(
                ps_out[:],
                lhsT=v_t[q][:],
                rhs=mcat16_sb[:, q, :],
                start=(q == 0),
                stop=False,
                skip_group_check=True,
            )
            nc.tensor.matmul(
                ps_out[:],
                lhsT=v8_t[q][:],
                rhs=mcat8_sb[:, q, :],
                start=False,
                stop=(q == NQ - 1),
                skip_group_check=True,
            )

        # ---- per-chunk pipeline, software-pipelined TWO chunks deep: by
        # the time stage2(q-2) is schedulable its V tiles are long built,
        # so the serial s1 -> DVE V-build -> s2 chain (measured ~1.9 us of
        # PE idle per chunk at one-deep) never gates the PE, wherever the
        # Tile scheduler places it.
        # idle-window fillers take a fake dep on the chunk's noise tile
        # (ready with the DMA, unlike the V chain) and are sized to the
        # DMA-schedule bubbles: the mcat halves ride between chunks 0-1
        # and 2-3, costing the PE ~2.1 us of idle each if unfilled.
        GAP_FILL = {0: 4, 1: 3, 2: 3, 3: 3, 4: 3, 5: 3, 6: 2, 7: 0}

        def gap_fill(q):
            for _ in range(GAP_FILL[q]):
                filler(dep=noise_t[q][:, 0, :])

        stage1_half(0, 0)
        stage1_half(0, 1)
        gap_fill(0)
        stage1_half(1, 0)
        stage1_half(1, 1)
        gap_fill(1)
        for q in range(2, NQ):
            stage1_half(q, 0)
            stage2(q - 2)
            stage1_half(q, 1)
            gap_fill(q)
        stage2(NQ - 2)
        stage2(NQ - 1)
        # keep the PE busy (clock warm) into the cast/store window and
        # the runtime's trailing barrier loop, which runs at the PE
        # sequencer's pace
        for _ in range(3):
            filler(dep=v8_t[NQ - 1][:])

        # ---- epilogue: one psum -> fp16 cast, split across ACT and DVE;
        # the host descales the x2^12 during the upcast ----
        out_sb = consts.tile([BSH, OUT], F16, tag="outsb")
        nc.scalar.copy(out_sb[:, 0 : OUT // 2], ps_out[:, 0 : OUT // 2])
        nc.vector.tensor_scalar_mul(out_sb[:, OUT // 2 : OUT],
                                    ps_out[:, OUT // 2 : OUT], 1.0)
        nc.sync.dma_start(out=out_d[:], in_=out_sb[:])

    _split_sync_waits(nc)
    _CACHE["nc"] = nc
    return nc


def _host_precompute(decay_param, conv_w, conv_b, out_w, out_b):
    dp = float(np.asarray(decay_param).reshape(-1)[0])
    decay = 0.5 / (1.0 + np.exp(-dp))
    dt = 1.0 / NS
    retain = 1.0 - decay * dt

    s = np.arange(NS, dtype=np.float64)
    gA = (1.0 - retain ** (NS - s)) / (1.0 - retain)
    gL = retain ** (NS - 1 - s)
    g2 = np.zeros((NS, 2), np.float32)
    g2[:, 0] = gA
    g2[:, 1] = gL

    conv_w = np.asarray(conv_w, np.float32)
    out_w = np.asarray(out_w, np.float32)
    w_sum = conv_w.sum(axis=2)
    t_sum = out_w @ w_sum              # [OUT, P]
    t0 = out_w @ conv_w[:, :, 0]
    r = np.stack([t_sum, -t0])         # [2, OUT, P]
    r_pad = np.zeros((2, OUT, PPAD), np.float32)
    r_pad[:, :, :P] = r
    rq = r_pad.reshape(2, OUT, NQ, 128).transpose(3, 2, 0, 1)  # [128, NQ, 2, OUT]
    bias_vec = (
        out_w @ np.asarray(conv_b, np.float32)
        + np.asarray(out_b, np.float32).reshape(OUT)
    )
    mcat16 = np.ascontiguousarray(rq[:, :, 0, :].astype(F16_NP))  # [128, NQ, OUT]
    mcat16[127, NQ - 1, :] = bias_vec.astype(F16_NP)  # bias rides padding row
    mcat8 = np.ascontiguousarray((rq[:, :, 1, :] * S8).astype(F8_NP))
    return g2, mcat16, mcat8


def kernel(x, noise, fb_w, fb_b, decay_param, conv_w, conv_b, out_w, out_b,
           _trace=False):
    global LAST_RUN

    x = np.asarray(x, np.float32)
    # chunk-major, feature-padded, per-core noise layout [core][q][s][b][i]:
    # every DMA descriptor reads an 8 KB sequential DRAM run.
    n16 = np.zeros((B, NS, PPAD), F16_NP)
    n16[:, :, :P] = np.asarray(noise, np.float32).astype(F16_NP)
    noise_q = np.ascontiguousarray(
        n16.reshape(NCORES, BSH, NS, NQ, 128).transpose(0, 3, 2, 1, 4)
    )  # [NCORES, NQ, NS, BSH, 128]

    g2, mcat16, mcat8 = _host_precompute(decay_param, conv_w, conv_b,
                                         out_w, out_b)

    # per-sample feedback scale: sigmoid(x . fb_w + fb_b) * sqrt(dt)/NS
    fb_w = np.asarray(fb_w, np.float32).reshape(IN)
    fb_b = float(np.asarray(fb_b, np.float32).reshape(-1)[0])
    z = x @ fb_w + fb_b
    cvec = (1.0 / (1.0 + np.exp(-z, dtype=np.float64))) * (np.sqrt(1.0 / NS) / NS)
    cvec = cvec.reshape(B).astype(np.float32)

    # bias rides mcat16[127, 7, :] (a feature-padding row): inject noise
    # values that make stage-1 x V-build produce V[127, A, b] = SA there.
    # The L-column side effect lands on mcat8's zero padding row.
    inj = (1.0 / (np.float64(g2[0, 0]) * cvec)).astype(F16_NP)  # [B]
    noise_q[:, NQ - 1, 0, :, 127] = inj.reshape(NCORES, BSH)

    nc = _build_program()

    in_maps = []
    for c in range(NCORES):
        sl = slice(c * BSH, (c + 1) * BSH)
        cblk = np.zeros((128, 256), np.float32)
        cblk[:, 2:34] = cvec[sl] * SA     # A columns carry the x2^12 scale
        cblk[:, 34:66] = cvec[sl] * SV    # L columns feed the fp8 V build
        cblk[:, 0:2] = g2
        in_maps.append(
            {
                "noise_sh": noise_q[c],
                "cblk": np.ascontiguousarray(cblk.astype(F16_NP)),
                "mcat16": mcat16,
                "mcat8": mcat8,
            }
        )

    res = run_bass_kernel_spmd(nc, in_maps, core_ids=list(range(NCORES)),
                               trace=_trace)
    LAST_RUN = res
    out = np.concatenate([m["out"] for m in res.results], axis=0)
    return out.astype(np.float32) * (1.0 / SA)


# revision 36
# speedup vs baseline: 1.0884x; 1.0884x over previous
"""Trainium2 Bass kernel for nn_BICEPNeuralLayer.

Math: the reference module (Euler-Maruyama SDE scan -> Conv1d over time ->
time-mean -> linear projection) is LINEAR in the noise tensor, so the whole
pipeline collapses algebraically:

  paths[t] = c_b * sum_s retain^(t-s) eps_s          (c_b = feedback_b*sqrt(dt))
  mean_t(conv(paths)) folds to per-timestep weights on eps:
     out[b] = (c_b/NS) * (Tsum @ A[b] - T0 @ L[b] - T2 @ F[b]) + bias
  A[b,i] = sum_s gA[s] noise[b,s,i],   gA[s] = (1-retain^(NS-s))/(1-retain)
  L[b,i] = sum_s retain^(NS-1-s) noise[b,s,i]
  F[b,i] = noise[b,0,i]
  Tsum = out_w @ (W0+W1+W2), T0 = out_w @ W0  (Wk = conv_w[:,:,k])
  bias  = out_w @ conv_b + out_b

The F term carries ~1e-6 of the output variance (Var A ~ 8e5, Var L ~ 1e2,
Var F ~ 1) and is dropped: ~1.1e-3 relative error against a 2e-2 gate, for
0.5 MB less HBM stream and 8 fewer matmuls per core. The bias vector rides
for free inside mcat16: chunk 7's rows 104-127 are feature padding
(P=1000 -> 1024), so row 127 of mcat16[:,7,:] holds the bias and the host
injects noise values that make the matching V row come out as the scale
constant - the chunk-7 A-matmul then adds the bias with no extra
instruction or transfer.

Scale unification: the A path carries x2^12 (folded into the c broadcast)
and the L path lands at the same 2^12 (= SV*S8/2^4... SV=16, S8=256), so
ps_out and ps_lf are directly addable: the whole epilogue is ONE DVE
tensor_add into fp16, and the host divides by 4096 during the upcast.

Device work per core (pure data parallel over batch, 32 samples/core):
  The noise shard is pre-transposed on the host to chunk-major layout
  [q][s][b][i] so every DMA descriptor is an 8 KB sequential DRAM run, and
  rides eight 1 MB transfers on the sync HWDGE queue (slicing transfers
  finer exposes a per-transfer completion-receipt stall of the SDMA
  engines - measured 16-130 GB/s on 256 KB b-sliced quarters vs 420 GB/s
  on 1 MB chunks). All weights (one [128,256] fp16 consts block + the two
  mcat halves each) stream concurrently on the scalar HWDGE queue, so
  their receipts hide under the noise stream and every weight is resident
  by ~14 us.

  HAM discipline: PE_HAM clock-gates the PE to 1.2 GHz after any ~3.4 us
  activity window containing idle. The PE is kept continuously busy with
  zero-operand filler matmuls: a pre-stream burst covering the first
  chunk's DMA window, and a few per chunk (the PE at full clock is ~1.8x
  faster than the noise stream).

  Stage-1 writes alternate full PSUM banks per half-chunk: the DVE V-build
  reads bank k while the PE writes bank k+1 (sharing one bank serialized
  the engines and cost ~0.5 us per half, measured).

  per chunk q (software-pipelined: s1(q,h0), s2(q-1), s1(q,h1)):
    stage 1: 16 matmuls per half, lhsT=noise[q][:,b,:] (fp16, FWL)
             rhs=g2[128,2] -> psum[i, (b,{A,L})]   (~33 ns/matmul)
    V build: DVE reorder (b,v)->(v,b) fused with the per-sample feedback
             scale (A column pre-scaled x2^12), per half
    ACT:     fp8 copy (x16) of the L columns, per half
    stage 2: A-term fp16 matmul into ps_out, L-term fp8xfp8 into ps_lf

  The output is stored fp16 at x4096 (host upcasts and descales).
"""

import sys

if "/opt/trn_rl_repo" not in sys.path:
    sys.path.insert(0, "/opt/trn_rl_repo")

from contextlib import ExitStack

import numpy as np

import concourse.bass as bass
import concourse.tile as tile
from concourse import mybir
from concourse.bass_utils import run_bass_kernel_spmd

B, IN, OUT, P, NS = 256, 1024, 512, 1000, 128
NCORES = 8
BSH = B // NCORES  # 32 samples per core
NQ = 8             # feature chunks of 128 (P padded 1000 -> 1024)
PPAD = NQ * 128
NPRE = 12          # pre-stream HAM warmup fillers (dependency-free)

F32 = mybir.dt.float32
F16 = mybir.dt.float16
F8 = mybir.dt.float8e4
F16_NP = mybir.dt.np(F16)
F8_NP = mybir.dt.np(F8)
S8 = 256.0         # fp8 T0 slice scale
SV = 16.0          # fp8 V(L) scale
SA = S8 * SV       # = 4096: A-path scale, folded into the c broadcast

_CACHE = {}

LAST_RUN = None  # BassKernelResults of the most recent execution (for test.py)


def _split_sync_waits(nc: bass.Bass, max_waits: int = 1) -> int:
    """Walrus in this container accepts at most one sync-wait command per
    instruction. Tile emits instructions (notably the epilogue Drain and any
    op depending on two DMA queues) with several waits. Split the surplus
    onto single-wait NoOps inserted just before, on the same engine, which
    is semantically identical for sem-ge waits."""
    nid = 0
    for fn in nc.m.functions:
        for bb in fn.blocks:
            insts = list(bb.instructions)
            out, changed = [], False
            for inst in insts:
                si = inst.sync_info
                if si is not None and si.on_wait and len(si.on_wait) > max_waits:
                    waits = list(si.on_wait)
                    extra, keep = waits[:-max_waits], waits[-max_waits:]
                    for w in extra:
                        nid += 1
                        out.append(
                            mybir.InstNoOp(
                                name=f"waitsplit-{nid}",
                                sync_info=mybir.SyncInfo(on_wait=[w], on_update=[]),
                                bass_nofuse=True,
                                engine=inst.engine,
                            )
                        )
                    inst.sync_info = mybir.SyncInfo(
                        on_wait=keep, on_update=list(si.on_update)
                    )
                    changed = True
                out.append(inst)
            if changed:
                bb.instructions = out
    return nid


def _build_program() -> bass.Bass:
    if "nc" in _CACHE:
        return _CACHE["nc"]

    nc = bass.Bass()

    noise_d = nc.dram_tensor("noise_sh", [NQ, NS, BSH, 128], F16,
                             kind="ExternalInput")
    # cols 0:2 g2 (gA, gL), 2:66 c broadcast in (v,b) layout, rest pad
    cblk_d = nc.dram_tensor("cblk", [128, 256], F16, kind="ExternalInput")
    mcat16_d = nc.dram_tensor("mcat16", [128, NQ, OUT], F16, kind="ExternalInput")
    mcat8_d = nc.dram_tensor("mcat8", [128, NQ, OUT], F8, kind="ExternalInput")
    out_d = nc.dram_tensor("out", [BSH, OUT], F16, kind="ExternalOutput")

    with ExitStack() as ctx:
        tc = ctx.enter_context(tile.TileContext(nc))
        consts = ctx.enter_context(tc.tile_pool(name="consts", bufs=1))
        npool = ctx.enter_context(tc.tile_pool(name="noise", bufs=NQ))
        vpool = ctx.enter_context(tc.tile_pool(name="v", bufs=1))
        ps1 = ctx.enter_context(tc.tile_pool(name="ps1", bufs=4, space="PSUM"))
        ps2 = ctx.enter_context(tc.tile_pool(name="ps2", bufs=1, space="PSUM"))
        wps = ctx.enter_context(tc.tile_pool(name="wps", bufs=1, space="PSUM"))

        # ---- tiles ----
        cblk_sb = consts.tile([128, 256], F16, tag="cblk")
        mcat16_sb = consts.tile([128, NQ, OUT], F16, tag="mcat16")
        mcat8_sb = consts.tile([128, NQ, OUT], F8, tag="mcat8")
        noise_t = [npool.tile([NS, BSH, 128], F16, name=f"noise{q}", tag="noise")
                   for q in range(NQ)]
        g2_sb = cblk_sb[:, 0:2]
        cbc_sb = cblk_sb[:, 2:66].rearrange("p (v b) -> p v b", v=2)

        # ---- DMA issues first, one queue, weights interleaved behind the
        # chunks that gate their first use (a second HWDGE queue only
        # splits the same HBM bandwidth, and it runs its transfers
        # markedly slower - measured ~100 GB/s - stretching the stream).
        nc.sync.dma_start(out=cblk_sb[:], in_=cblk_d[:])
        nc.sync.dma_start(out=noise_t[0][:], in_=noise_d[0])
        nc.sync.dma_start(out=mcat16_sb[:, 0:4, :], in_=mcat16_d[:][:, 0:4, :])
        nc.sync.dma_start(out=mcat8_sb[:, 0:4, :], in_=mcat8_d[:][:, 0:4, :])
        nc.sync.dma_start(out=noise_t[1][:], in_=noise_d[1])
        nc.sync.dma_start(out=noise_t[2][:], in_=noise_d[2])
        nc.sync.dma_start(out=mcat16_sb[:, 4:8, :], in_=mcat16_d[:][:, 4:8, :])
        nc.sync.dma_start(out=mcat8_sb[:, 4:8, :], in_=mcat8_d[:][:, 4:8, :])
        for q in range(3, NQ):
            nc.sync.dma_start(out=noise_t[q][:], in_=noise_d[q])

        # ---- HAM warmup/filler scaffolding: zero fp8 operands, scratch
        # psum bank. The pre-burst is dependency-free (the scheduler runs
        # it first); per-chunk fillers take a fake RAW dep on v8_t[q] so
        # the priority-heap scheduler slots them exactly into the PE's
        # per-chunk idle window (the PE outruns the noise stream ~1.5x,
        # and any idle re-throttles the PE clock to 1.2 GHz, doubling the
        # N=512 stage-2 matmuls and the runtime's epilogue barrier loop).
        warm_sb = consts.tile([128, 512], F16, tag="warm")
        nc.vector.memset(warm_sb[:], 0.0)
        warm_ps = wps.tile([128, 512], F32, tag="warmps")

        def filler(dep=None):
            lhsT = dep if dep is not None else warm_sb[:, 0:128]
            nc.tensor.matmul(warm_ps[0 : lhsT.free_size(), :], lhsT=lhsT,
                             rhs=warm_sb[:], start=True, stop=True)

        for _ in range(NPRE):
            filler()

        ps_out = ps2.tile([BSH, OUT], F32, tag="ps2")
        v_t = [vpool.tile([128, BSH], F16, name=f"v{q}", tag=f"v{q}")
               for q in range(NQ)]
        v8_t = [vpool.tile([128, BSH], F8, name=f"v8_{q}", tag=f"v8_{q}")
                for q in range(NQ)]
        HW_ = BSH // 2
        # one full PSUM bank per half-chunk (writing the bank the DVE is
        # still reading serializes PE vs DVE)
        pt_t = [ps1.tile([128, 512], F32, name=f"ps1_{qh}", tag="ps1")
                for qh in range(2 * NQ)]

        def stage1_half(q, h):
            pt = pt_t[2 * q + h]
            for j, b in enumerate(range(h * HW_, (h + 1) * HW_)):
                nc.tensor.matmul(
                    pt[:, j * 2 : j * 2 + 2],
                    lhsT=noise_t[q][:, b, :],
                    rhs=g2_sb,
                    start=True,
                    stop=True,
                )
            # psum -> V: two DVE ops straight off the psum, folding the
            # per-sample feedback scale c_b in. A path fp16 (x2^12 via the
            # c broadcast), L path fp8 (x16 via its own c columns).
            bs = slice(h * HW_, (h + 1) * HW_)
            src = pt[:, 0 : 2 * HW_].rearrange("p (b v) -> p v b", v=2)
            nc.vector.tensor_mul(v_t[q][:, bs], src[:, 0, :], cbc_sb[:, 0, bs])
            nc.vector.tensor_mul(v8_t[q][:, bs], src[:, 1, :], cbc_sb[:, 1, bs])

        def stage2(q):
            # both terms land at x2^12, so the L matmuls accumulate
            # straight into ps_out (fp8 and fp16 matmuls share the f32
            # accumulation); A first - it only needs the first DVE op
            nc.tensor.matmul(
                ps_out[:],
                lhsT=v_t[q][:],
                rhs=mcat16_sb[:, q, :],
                start=(q == 0),
                stop=False,
                skip_group_check=True,
            )
            nc.tensor.matmul(
                ps_out[:],
                lhsT=v8_t[q][:],
                rhs=mcat8_sb[:, q, :],
                start=False,
                stop=(q == NQ - 1),
                skip_group_check=True,
            )

        # ---- per-chunk pipeline, software-pipelined TWO chunks deep: by
        # the time stage2(q-2) is schedulable its V tiles are long built,
        # so the serial s1 -> DVE V-build -> s2 chain (measured ~1.9 us of
        # PE idle per chunk at one-deep) never gates the PE, wherever the
        # Tile scheduler places it.
        # idle-window fillers take a fake dep on the chunk's noise tile
        # (ready with the DMA, unlike the V chain) and are sized to the
        # DMA-schedule bubbles: the mcat halves ride between chunks 0-1
        # and 2-3, costing the PE ~2.1 us of idle each if unfilled.
        GAP_FILL = {0: 15, 1: 4, 2: 10, 3: 3, 4: 2, 5: 2, 6: 2, 7: 0}

        def gap_fill(q):
            for _ in range(GAP_FILL[q]):
                filler(dep=noise_t[q][:, 0, :])

        stage1_half(0, 0)
        stage1_half(0, 1)
        gap_fill(0)
        stage1_half(1, 0)
        stage1_half(1, 1)
        gap_fill(1)
        for q in range(2, NQ):
            stage1_half(q, 0)
            stage2(q - 2)
            stage1_half(q, 1)
            gap_fill(q)
        stage2(NQ - 2)
        stage2(NQ - 1)
        # keep the PE busy (clock warm) into the cast/store window and
        # the runtime's trailing barrier loop, which runs at the PE
        # sequencer's pace
        for _ in range(3):
            filler(dep=v8_t[NQ - 1][:])

        # ---- epilogue: one psum -> fp16 cast, split across ACT and DVE;
        # the host descales the x2^12 during the upcast ----
        out_sb = consts.tile([BSH, OUT], F16, tag="outsb")
        nc.scalar.copy(out_sb[:, 0 : OUT // 2], ps_out[:, 0 : OUT // 2])
        nc.vector.tensor_scalar_mul(out_sb[:, OUT // 2 : OUT],
                                    ps_out[:, OUT // 2 : OUT], 1.0)
        nc.sync.dma_start(out=out_d[:], in_=out_sb[:])

    _split_sync_waits(nc)
    _CACHE["nc"] = nc
    return nc


def _host_precompute(decay_param, conv_w, conv_b, out_w, out_b):
    dp = float(np.asarray(decay_param).reshape(-1)[0])
    decay = 0.5 / (1.0 + np.exp(-dp))
    dt = 1.0 / NS
    retain = 1.0 - decay * dt

    s = np.arange(NS, dtype=np.float64)
    gA = (1.0 - retain ** (NS - s)) / (1.0 - retain)
    gL = retain ** (NS - 1 - s)
    g2 = np.zeros((NS, 2), np.float32)
    g2[:, 0] = gA
    g2[:, 1] = gL

    conv_w = np.asarray(conv_w, np.float32)
    out_w = np.asarray(out_w, np.float32)
    w_sum = conv_w.sum(axis=2)
    t_sum = out_w @ w_sum              # [OUT, P]
    t0 = out_w @ conv_w[:, :, 0]
    r = np.stack([t_sum, -t0])         # [2, OUT, P]
    r_pad = np.zeros((2, OUT, PPAD), np.float32)
    r_pad[:, :, :P] = r
    rq = r_pad.reshape(2, OUT, NQ, 128).transpose(3, 2, 0, 1)  # [128, NQ, 2, OUT]
    bias_vec = (
        out_w @ np.asarray(conv_b, np.float32)
        + np.asarray(out_b, np.float32).reshape(OUT)
    )
    mcat16 = np.ascontiguousarray(rq[:, :, 0, :].astype(F16_NP))  # [128, NQ, OUT]
    mcat16[127, NQ - 1, :] = bias_vec.astype(F16_NP)  # bias rides padding row
    mcat8 = np.ascontiguousarray((rq[:, :, 1, :] * S8).astype(F8_NP))
    return g2, mcat16, mcat8


def kernel(x, noise, fb_w, fb_b, decay_param, conv_w, conv_b, out_w, out_b,
           _trace=False):
    global LAST_RUN

    x = np.asarray(x, np.float32)
    # chunk-major, feature-padded, per-core noise layout [core][q][s][b][i]:
    # every DMA descriptor reads an 8 KB sequential DRAM run.
    n16 = np.zeros((B, NS, PPAD), F16_NP)
    n16[:, :, :P] = np.asarray(noise, np.float32).astype(F16_NP)
    noise_q = np.ascontiguousarray(
        n16.reshape(NCORES, BSH, NS, NQ, 128).transpose(0, 3, 2, 1, 4)
    )  # [NCORES, NQ, NS, BSH, 128]

    g2, mcat16, mcat8 = _host_precompute(decay_param, conv_w, conv_b,
                                         out_w, out_b)

    # per-sample feedback scale: sigmoid(x . fb_w + fb_b) * sqrt(dt)/NS
    fb_w = np.asarray(fb_w, np.float32).reshape(IN)
    fb_b = float(np.asarray(fb_b, np.float32).reshape(-1)[0])
    z = x @ fb_w + fb_b
    cvec = (1.0 / (1.0 + np.exp(-z, dtype=np.float64))) * (np.sqrt(1.0 / NS) / NS)
    cvec = cvec.reshape(B).astype(np.float32)

    # bias rides mcat16[127, 7, :] (a feature-padding row): inject noise
    # values that make stage-1 x V-build produce V[127, A, b] = SA there.
    # The L-column side effect lands on mcat8's zero padding row.
    inj = (1.0 / (np.float64(g2[0, 0]) * cvec)).astype(F16_NP)  # [B]
    noise_q[:, NQ - 1, 0, :, 127] = inj.reshape(NCORES, BSH)

    nc = _build_program()

    in_maps = []
    for c in range(NCORES):
        sl = slice(c * BSH, (c + 1) * BSH)
        cblk = np.zeros((128, 256), np.float32)
        cblk[:, 2:34] = cvec[sl] * SA     # A columns carry the x2^12 scale
        cblk[:, 34:66] = cvec[sl] * SV    # L columns feed the fp8 V build
        cblk[:, 0:2] = g2
        in_maps.append(
            {
                "noise_sh": noise_q[c],
                "cblk": np.ascontiguousarray(cblk.astype(F16_NP)),
                "mcat16": mcat16,
                "mcat8": mcat8,
            }
        )

    res = run_bass_kernel_spmd(nc, in_maps, core_ids=list(range(NCORES)),
                               trace=_trace)
    LAST_RUN = res
    out = np.concatenate([m["out"] for m in res.results], axis=0)
    return out.astype(np.float32) * (1.0 / SA)
